# revision 48
# baseline (speedup 1.0000x reference)
"""Multi-head attention (B=2, S=2048, EMB=1024, H=16, hd=64) on 8 TRN2 cores.

Sharding: core c -> batch b = c//4, head-group g = c%4 (4 heads, 256 emb dims).
All matmuls in bf16 (1 cyc/row full-rate streaming, ~6e-3 rel err).
Per core:
  A) Q^T = Wq_g @ x_b^T, K^T = Wk_g @ x_b^T  [256, 2048] (transposed layout)
     (V's projection is deferred into phase B's first quarter)
  B) quarter pipeline over (head-pair mh, q-half qh): scores S^T[k,q] =
     K_h @ Q_h^T stream into a 4-bank psum ping-pong; P^T = exp(S^T/8) on
     Act (the pacing engine, ~1.2us per [128,1024] tile); each quarter's
     U_aug[65, SH] += [V_h|1].T @ P^T matmuls are pumped from a FIFO into
     the NEXT quarter's exp window (2 units/step), with a 16-deep p-tile
     ring so Act never stalls.  V's matmuls are the pumped work during
     quarter 0 (its psum lives in a 2-bank pool stacked above the score
     banks; the U accumulators inherit those banks afterwards).
  C) per (pair, q-half): r = 1/sums (row 64 of U_aug; DVE recip approx),
     broadcast over 64 partitions via a DRAM-bounce DMA, O^T = U^T * r
     overwrites qT.  Only the last quarter's chain is tail-exposed.
  D) y = O @ Wo_g^T [2048, 1024] with both head-pairs accumulated in psum,
     cast to bf16 (alternating Act/DVE), per-tile DMA out; host sums the
     4 head-group partials in f32.
Weights arrive host-pre-arranged so every weight DMA is contiguous.
"""
import numpy as np

import concourse.bass as bass
import concourse.tile as tile
from concourse import bacc, mybir
from concourse.bass_utils import run_bass_kernel_spmd

import os

F32 = mybir.dt.float32
F32R = mybir.dt.float32r
BF16 = mybir.dt.bfloat16
FP16 = mybir.dt.float16
# matmul dtype: f32r (2 cyc/row, ~4e-4) | fp16 (1 cyc/row, ~1e-3) | bf16
MM_DT_NAME = os.environ.get("MM_DT", "bf16")
MM = {"f32r": F32R, "bf16": BF16, "fp16": FP16}[MM_DT_NAME]
IN_DT = {"f32r": F32, "bf16": BF16, "fp16": FP16}[MM_DT_NAME]
EXP = mybir.ActivationFunctionType.Exp
MULT = mybir.AluOpType.mult

EMB = 1024
S = 2048
B = 2
HG = 4           # heads per core
HD = 64
CHD = HG * HD    # 256 emb dims per core
ET = EMB // 128  # 8 e-tiles
NT = S // 128    # 16 s/k-tiles
QB = 512
NQB = S // QB    # 4

_NC = None


def _mm(ap):
    """View a dram input AP with the matmul dtype (bitcast only for f32r)."""
    return ap.bitcast(F32R) if MM == F32R else ap


def _build(dbg=False):
    nc = bacc.Bacc("TRN2", target_bir_lowering=False, debug=False)
    xq_t = nc.dram_tensor("xq_t", [EMB, S], IN_DT, kind="ExternalInput").ap()
    xk_t = nc.dram_tensor("xk_t", [EMB, S], IN_DT, kind="ExternalInput").ap()
    xv_t = nc.dram_tensor("xv_t", [EMB, S], IN_DT, kind="ExternalInput").ap()
    # weights arrive pre-arranged from the host so every weight DMA is a
    # single contiguous read per partition (the old "(po pi) m -> pi po m"
    # gather cost ~12us and gated the whole phase A)
    wq_t = nc.dram_tensor("wq_t", [128, ET * CHD], IN_DT,
                          kind="ExternalInput").ap()
    wk_t = nc.dram_tensor("wk_t", [128, ET * CHD], IN_DT,
                          kind="ExternalInput").ap()
    wv_t = nc.dram_tensor("wv_t", [128, ET * CHD], IN_DT,
                          kind="ExternalInput").ap()
    wo_t = nc.dram_tensor("wo_t", [128, 2 * EMB], IN_DT,
                          kind="ExternalInput").ap()
    y = nc.dram_tensor("y", [S, EMB], BF16, kind="ExternalOutput").ap()
    if dbg:
        dbg_qT = nc.dram_tensor("dbg_qT", [128, 2, S], F32, kind="ExternalOutput").ap()
        dbg_kT = nc.dram_tensor("dbg_kT", [128, 2, S], F32, kind="ExternalOutput").ap()
        dbg_v = nc.dram_tensor("dbg_v", [128, NT, HG * (HD + 1)], F32,
                               kind="ExternalOutput").ap()
        dbg_u = nc.dram_tensor("dbg_u", [HG, HD + 1, S], F32,
                               kind="ExternalOutput").ap()
        dbg_r = nc.dram_tensor("dbg_r", [HG, S], F32, kind="ExternalOutput").ap()
        dbg_oT = nc.dram_tensor("dbg_oT", [128, 2, S], F32, kind="ExternalOutput").ap()

    with tile.TileContext(nc) as tc:
        with tc.tile_pool(name="const", bufs=1) as cpool, \
             tc.tile_pool(name="wqk", bufs=2) as wpool, \
             tc.tile_pool(name="big", bufs=1) as big, \
             tc.tile_pool(name="usb", bufs=4) as usb, \
             tc.tile_pool(name="xp", bufs=8) as xp, \
             tc.tile_pool(name="pt", bufs=2) as ptp, \
             tc.tile_pool(name="yp", bufs=2) as ypool, \
             tc.tile_pool(name="rp", bufs=2) as rpool, \
             tc.tile_pool(name="rd", bufs=4, space="DRAM") as rdram:

            # ---- static weights (wo DMA deferred past phase A) ----
            wo_sb = cpool.tile([128, 2, EMB], MM, name="wo_sb")

            qT = big.tile([128, 2, S], MM, name="qT")     # later reused as O^T
            kT = big.tile([128, 2, S], MM, name="kT")
            v_sb = big.tile([128, NT, HG * (HD + 1)], MM, name="v_sb")
            if MM == F32R:
                nc.vector.memset(v_sb[:].bitcast(F32), 1.0)
            else:
                nc.vector.memset(v_sb[:], 1.0)     # ones cols survive

            # ---- phase A: projections ----
            # Q: both head-pairs.  K: pair-0 only — pair-1's K matmuls are
            # pumped through the phase-B FIFO (like V), so the exp stream
            # starts ~7us earlier (it only needs qT and kT[:,0]).
            xk_tiles = []
            wk_sb = None
            with tc.tile_pool(name="psA", bufs=8, space="PSUM") as psA:
                for name, xdram, wdram, dst, ms in (
                        ("q", xq_t, wq_t, qT, (0, 1)),
                        ("k", xk_t, wk_t, kT, (0,))):
                    w_sb = wpool.tile([128, ET, CHD], MM, tag="w",
                                      name=f"w{name}_sb")
                    nc.sync.dma_start(
                        w_sb[:],
                        _mm(wdram).rearrange("p (e m) -> p e m", m=CHD))
                    if name == "k":
                        wk_sb = w_sb
                    pss = [psA.tile([128, QB], F32, tag="ps", name=f"ps_{name}{i}")
                           for i in range(4 * len(ms))]
                    for e in range(ET):
                        x_t = xp.tile([128, S], MM, tag="x", name=f"x_{name}{e}")
                        nc.sync.dma_start(
                            x_t[:], _mm(xdram)[e * 128:(e + 1) * 128, :])
                        if name == "k":
                            xk_tiles.append(x_t)
                        for mi, m in enumerate(ms):
                            for qb in range(NQB):
                                nc.tensor.matmul(
                                    pss[mi * NQB + qb][:],
                                    w_sb[:, e, m * 128:(m + 1) * 128],
                                    x_t[:, qb * QB:(qb + 1) * QB],
                                    start=(e == 0), stop=(e == ET - 1))
                    for mi, m in enumerate(ms):
                        for qb in range(NQB):
                            cp = nc.scalar.copy if (m + qb) % 2 else \
                                nc.vector.tensor_copy
                            cp(dst[:, m, qb * QB:(qb + 1) * QB],
                               pss[mi * NQB + qb][:])

            # V is NOT projected here: its matmuls interleave with B pair-0's
            # first q-half so the PE fills the Act-paced exp window.  Only
            # the DMAs are issued now (they ride behind xq/xk in the queue).
            wv_sb = wpool.tile([128, ET, CHD], MM, tag="w", name="wv_sb")
            nc.sync.dma_start(
                wv_sb[:],
                _mm(wv_t).rearrange("p (e m) -> p e m", m=CHD))
            xv_tiles = []
            for e in range(ET):
                x_t = xp.tile([128, S], MM, tag="x", name=f"x_v{e}")
                nc.sync.dma_start(
                    x_t[:], _mm(xv_t)[e * 128:(e + 1) * 128, :])
                xv_tiles.append(x_t)
            # deferred weight loads ride behind the xv DMAs
            nc.sync.dma_start(
                wo_sb[:], _mm(wo_t).rearrange("p (ct n) -> p ct n", n=EMB))

            # ---- phase B: attention, head-PAIRS packed on PE ----
            # Heads 2mh (rows 0-63) and 2mh+1 (rows 64-127) issue scores
            # matmuls into different PE row-groups + different psum banks, so
            # they run concurrently. q is split in halves so both heads'
            # U accumulators fit PSUM ([65, 1024] = 2 banks each).
            u_list = [None] * HG
            SH = S // 2

            def emit_scores(mh, qh, t, psS):
                qo = qh * SH
                spA = psS.tile([128, SH], F32, tag="spsA",
                               name=f"spsA{mh}{qh}{t}")
                spB = psS.tile([128, SH], F32, tag="spsB",
                               name=f"spsB{mh}{qh}{t}")
                for j in range(2):
                    for bp, sp in ((0, spA), (64, spB)):
                        nc.tensor.matmul(
                            sp[:, j * QB:(j + 1) * QB],
                            kT[bp:bp + HD, mh, t * 128:(t + 1) * 128],
                            qT[bp:bp + HD, mh,
                               qo + j * QB:qo + (j + 1) * QB],
                            start=True, stop=True)
                pA = ptp.tile([128, SH], MM, tag="ptA",
                              name=f"ptA{mh}{qh}{t}", bufs=16)
                nc.scalar.activation(pA[:], spA[:], EXP, scale=0.125)
                pB = ptp.tile([128, SH], MM, tag="ptB",
                              name=f"ptB{mh}{qh}{t}", bufs=16)
                nc.scalar.activation(pB[:], spB[:], EXP, scale=0.125)
                return pA, pB

            def emit_u(hA, hB, uaccA, uaccB, pA, pB, t):
                for h2, uacc, p_t in ((hA, uaccA, pA), (hB, uaccB, pB)):
                    for j in range(2):
                        nc.tensor.matmul(
                            uacc[:, j * QB:(j + 1) * QB],
                            v_sb[:, t, h2 * (HD + 1):(h2 + 1) * (HD + 1)],
                            p_t[:, j * QB:(j + 1) * QB],
                            start=(t == 0), stop=(t == NT - 1))

            def emit_norm(mh, qh, hA, hB, uA, uB, uaccA, uaccB):
                qo = qh * SH
                # drain both U accumulators on DVE (keeps Act free for the
                # next q-half's exp stream); for the final quarter Act is
                # already idle, so split the drains across both engines —
                # the psU pool release (and with it phase D) waits on them
                nc.vector.tensor_copy(uA[:, qo:qo + SH], uaccA[:])
                cpB = nc.scalar.copy if (mh, qh) == (1, 1) else                     nc.vector.tensor_copy
                cpB(uB[:, qo:qo + SH], uaccB[:])
                # per-q-half softmax normalization: r = 1/sums, broadcast r
                # across 64 partitions via a DRAM-bounce DMA (stride-0
                # partition reads are legal from DRAM), then O^T = U^T * r
                # overwrites qT.  Only the last quarter's chain is exposed
                # in the kernel tail.
                rp2 = rpool.tile([2, SH], F32, tag="rh",
                                 name=f"rp2_{mh}{qh}")
                nc.sync.dma_start(rp2[0:1, :], uA[HD:HD + 1, qo:qo + SH])
                nc.sync.dma_start(rp2[1:2, :], uB[HD:HD + 1, qo:qo + SH])
                r2 = rpool.tile([2, SH], F32, tag="rh2",
                                name=f"r2_{mh}{qh}")
                nc.vector.reciprocal_approx_fast(out=r2[:], in_=rp2[:])
                for h2, u_h in ((hA, uA), (hB, uB)):
                    bp2 = 64 * (h2 % 2)
                    rd = rdram.tile([1, SH], F32, name=f"rd{h2}_{qh}")
                    nc.sync.dma_start(rd[:], r2[h2 % 2:h2 % 2 + 1, :])
                    rb = rpool.tile([HD, SH], F32, tag="rb",
                                    name=f"rb{h2}_{qh}", bufs=3)
                    nc.sync.dma_start(rb[:], rd[:].to_broadcast([HD, SH]))
                    nc.vector.tensor_tensor(
                        qT[bp2:bp2 + HD, mh, qo:qo + SH],
                        u_h[0:HD, qo:qo + SH],
                        rb[:], MULT)

            # Quarter pipeline: quarter i's scores+exps run at Act pace
            # while the PE pumps quarter i-1's U matmuls (and, in quarter
            # 0, V's projection matmuls) at 2 units/step from a FIFO.  The
            # 16-deep p-tile ring lets the exp stream run a full quarter
            # ahead of U consumption, so Act never stalls.
            quarters = [(0, 0), (0, 1), (1, 0), (1, 1)]
            plists = {}
            fifo = []

            def pump(n):
                for _ in range(min(n, len(fifo))):
                    fifo.pop(0)()

            def make_v_unit(s):
                def unit():
                    v_ps = psA2_box[0].tile([128, CHD], F32, tag="psv",
                                            name=f"ps_v{s}")
                    for e in range(ET):
                        nc.tensor.matmul(
                            v_ps[:], xv_tiles[e][:, s * 128:(s + 1) * 128],
                            wv_sb[:, e, :],
                            start=(e == 0), stop=(e == ET - 1))
                    src_ = v_ps[:].rearrange("p (h d) -> p h d", d=HD)
                    dstv = v_sb[:, s, :].rearrange(
                        "p (h d) -> p h d", d=HD + 1)[:, :, 0:HD]
                    nc.vector.tensor_copy(dstv, src_)
                return unit

            def make_u_unit(hA, hB, uaccA, uaccB, plkey, t):
                def unit():
                    pA, pB = plists[plkey][t]
                    emit_u(hA, hB, uaccA, uaccB, pA, pB, t)
                return unit

            def queue_prev_quarter(psU, pmh, pqh):
                # build quarter (pmh,pqh)'s U units + its drain/norm tail
                phA, phB = 2 * pmh, 2 * pmh + 1
                uaccA = psU.tile([HD + 1, SH], F32, tag="uaccA",
                                 name=f"uaccA{pmh}_{pqh}")
                uaccB = psU.tile([HD + 1, SH], F32, tag="uaccB",
                                 name=f"uaccB{pmh}_{pqh}")
                for t in range(NT):
                    fifo.append(make_u_unit(phA, phB, uaccA, uaccB,
                                            (pmh, pqh), t))
                uA, uB = u_list[phA], u_list[phB]
                fifo.append(lambda: emit_norm(pmh, pqh, phA, phB, uA, uB,
                                              uaccA, uaccB))

            psA2_box = [None]
            with tc.tile_pool(name="psS", bufs=1, space="PSUM") as psS:
                # quarter (0,0): scores+exps, V pumped from the FIFO
                u_list[0] = usb.tile([HD + 1, S], F32, tag="u", name="u0")
                u_list[1] = usb.tile([HD + 1, S], F32, tag="u", name="u1")
                with tc.tile_pool(name="psA2", bufs=2, space="PSUM") as psA2:
                    psA2_box[0] = psA2
                    # K pair-1 units first (banks 6-7, resident xk tiles);
                    # V afterwards — its xv buffers recycle xk's, so the
                    # xv DMAs land just as the first v unit pops.
                    km1_box = [[None], [None]]

                    def make_km1_unit(p, e):
                        def unit():
                            if e == 0:
                                km1_box[p][0] = [
                                    psA2_box[0].tile([128, QB], F32,
                                                     tag="ps1",
                                                     name=f"ps_k1_{p}{i}")
                                    for i in range(2)]
                            for i in range(2):
                                qb = 2 * p + i
                                nc.tensor.matmul(
                                    km1_box[p][0][i][:],
                                    wk_sb[:, e, 128:256],
                                    xk_tiles[e][:, qb * QB:(qb + 1) * QB],
                                    start=(e == 0), stop=(e == ET - 1))
                        return unit

                    def make_km1_drain(p):
                        def unit():
                            for i in range(2):
                                qb = 2 * p + i
                                nc.vector.tensor_copy(
                                    kT[:, 1, qb * QB:(qb + 1) * QB],
                                    km1_box[p][0][i][:])
                        return unit

                    for p in range(2):
                        fifo.extend(make_km1_unit(p, e) for e in range(ET))
                        fifo.append(make_km1_drain(p))
                    fifo.extend(make_v_unit(s) for s in range(NT))
                    plists[(0, 0)] = []
                    for t in range(NT):
                        plists[(0, 0)].append(emit_scores(0, 0, t, psS))
                        # 3/step drains the cheap K units (0.43us) first;
                        # V units (0.96us) then go at an average 1.5/step
                        # to stay under the 1.53us/step PE budget the
                        # Act-paced exp cadence allows
                        pump(3 if t < 6 else (2 if t % 2 else 1))
                    pump(len(fifo))
                with tc.tile_pool(name="psU", bufs=1, space="PSUM") as psU:
                    for mh, qh in quarters[1:]:
                        if qh == 0:
                            hA, hB = 2 * mh, 2 * mh + 1
                            u_list[hA] = usb.tile([HD + 1, S], F32, tag="u",
                                                  name=f"u{hA}")
                            u_list[hB] = usb.tile([HD + 1, S], F32, tag="u",
                                                  name=f"u{hB}")
                        qi = quarters.index((mh, qh))
                        queue_prev_quarter(psU, *quarters[qi - 1])
                        plists[(mh, qh)] = []
                        for t in range(NT):
                            plists[(mh, qh)].append(
                                emit_scores(mh, qh, t, psS))
                            pump(2)
                        pump(len(fifo))
                    # final quarter's U work + norm run in the tail
                    queue_prev_quarter(psU, 1, 1)
                    pump(len(fifo))

            if dbg:
                nc.sync.dma_start(dbg_qT, qT[:].bitcast(F32)) if MM == F32R else None
                nc.sync.dma_start(dbg_kT, kT[:].bitcast(F32)) if MM == F32R else None
                nc.sync.dma_start(dbg_v, v_sb[:].bitcast(F32)) if MM == F32R else None
                for h in range(HG):
                    nc.sync.dma_start(dbg_u[h], u_list[h][:])

            # ---- phase D: output projection (qT now holds O^T) ----
            # ct accumulates in psum; one bf16 cast-copy per s-tile, split
            # between DVE and Act (both idle here), then a per-tile DMA so
            # the output drain overlaps the remaining matmuls.  s-tiles 0-7
            # only need the qh=0 halves of O^T, so the scheduler can start
            # them while the last q-half normalization is still in flight.
            with tc.tile_pool(name="psY", bufs=4, space="PSUM") as psY:
                for s in range(NT):
                    y_ps = psY.tile([128, EMB], F32, tag="yps",
                                    name=f"yps{s}")
                    for nb in range(2):
                        for ct in range(2):
                            nc.tensor.matmul(
                                y_ps[:, nb * QB:(nb + 1) * QB],
                                qT[:, ct, s * 128:(s + 1) * 128],
                                wo_sb[:, ct, nb * QB:(nb + 1) * QB],
                                start=(ct == 0), stop=(ct == 1))
                    y_out = ypool.tile([128, EMB], BF16, tag="yout",
                                       name=f"yout{s}", bufs=6)
                    cp = nc.scalar.copy if s % 2 else nc.vector.tensor_copy
                    cp(y_out[:], y_ps[:])
                    nc.sync.dma_start(y[s * 128:(s + 1) * 128, :], y_out[:])

    nc.compile()
    return nc


def get_nc():
    global _NC
    if _NC is None:
        _NC = _build()
    return _NC


def make_in_maps(query, key, value, Wq, Wk, Wv, Wo):
    import ml_dtypes
    np_dt = {F32R: np.float32, BF16: ml_dtypes.bfloat16,
             FP16: np.float16}[MM]
    query = np.asarray(query, dtype=np.float32)
    key = np.asarray(key, dtype=np.float32)
    value = np.asarray(value, dtype=np.float32)
    Wq = np.asarray(Wq, dtype=np.float32)
    Wk = np.asarray(Wk, dtype=np.float32)
    Wv = np.asarray(Wv, dtype=np.float32)
    Wo = np.asarray(Wo, dtype=np.float32)
    xt = {(n, b): np.ascontiguousarray(x[b].T).astype(np_dt)
          for n, x in (("q", query), ("k", key), ("v", value))
          for b in range(B)}
    def warr(w):
        # [EMB, CHD] -> [128, ET*CHD]: element [pi, po*CHD+m] = w[po*128+pi, m]
        return np.ascontiguousarray(
            w.reshape(8, 128, CHD).transpose(1, 0, 2).reshape(128, 8 * CHD)
        ).astype(np_dt)

    def woarr(w):
        # [CHD, EMB] -> [128, 2*EMB]: element [p, ct*EMB+n] = w[ct*128+p, n]
        return np.ascontiguousarray(
            w.reshape(2, 128, EMB).transpose(1, 0, 2).reshape(128, 2 * EMB)
        ).astype(np_dt)

    in_maps = []
    for c in range(8):
        b, g = divmod(c, 4)
        hs = slice(g * CHD, (g + 1) * CHD)
        in_maps.append({
            "xq_t": xt[("q", b)],
            "xk_t": xt[("k", b)],
            "xv_t": xt[("v", b)],
            "wq_t": warr(np.ascontiguousarray(Wq[hs, :].T)),
            "wk_t": warr(np.ascontiguousarray(Wk[hs, :].T)),
            "wv_t": warr(np.ascontiguousarray(Wv[hs, :].T)),
            "wo_t": woarr(np.ascontiguousarray(Wo[:, hs].T)),
        })
    return in_maps


def gather(results):
    out = np.zeros((B, S, EMB), dtype=np.float32)
    for c in range(8):
        out[c // 4] += results[c]["y"].astype(np.float32)
    return out


def kernel(**inputs) -> np.ndarray:
    nc = get_nc()
    in_maps = make_in_maps(**inputs)
    res = run_bass_kernel_spmd(nc, in_maps, core_ids=list(range(8)))
    return gather(res.results)

